# revision 87
# baseline (speedup 1.0000x reference)
"""Trainium2 Bass kernel for nn_LogisticMixture (discretized logistic mixture loss).

Contract: kernel(**inputs) takes FULL unsharded numpy inputs
  x      [128, 32, 32, 256] f32
  value  [128, 32, 32, 3]   f32 (integer pixel values 0..255)
  W_conv [256, 100]         f32
  b_conv [100]              f32
and returns the full [128] f32 output (per-image sum of mixture log-probs).

Strategy: pure data parallelism over batch across 8 NeuronCores (16384
pixels/core). Host pre-transposes x to tile-contiguous x^T blocks (bf16)
and PRESCALES W columns (locs *= -127.5, coeffs *= -1) so the PE emits
epilogue-ready params. The matmul + PSUM reads run per 2048-px macro-tile;
all remaining epilogue stages run once per PAIR of macro-tiles at double
width (fewer, fatter ops -> less per-instruction overhead and semaphore
traffic). Stages are software-pipelined and emitted deepest-first so every
cross-engine dependency is >=1 tile-iteration old. Math per (pixel,k,c),
all tensors c-major packed so every ACT/DVE op reads AND writes packed:

  A_c   = vp2_c + locs'_c (+ q*coeff' couplings)        [vp2 = value-127]
  ls    = ln(1+e^s) ; r16 = exp(-ln(ls)-ln(127.5)) on ACT (eps dropped)
  p0 = A*r16 ; nm0 = r16 - p0  (negated min_in)
  masked-shift: one +30000 aux add kills u/v/L2 terms for y==0/255 exactly
  ONE exp over [|p'|,|nm'|,r] (|x| = f16 sign-bit AND), gm*e^{-r} with
  host gm in {-1,0}, ONE ln(1+x) -> [L0,L1,L2]; u+v = L01 - min(pm',0)
  w     = u + v - L2 ; S = sum_c w (two adds, c-major blocks)
  mix   = lse_k(logits - S) - lse_k(logits)             -> acc per image

Engine split: DVE all f16 2x/4x-mode elementwise + reduces; ACT all
exp/ln (single natural_log_exp_and_others table load) + PSUM copies;
GpSimd (Pool) the coupling products/adds; PE the 1x1-conv matmuls.
"""
import sys
import os

for _p in ("/opt/trn_rl_repo", "/root/.axon_site/_ro/trn_rl_repo"):
    if os.path.isdir(_p) and _p not in sys.path:
        sys.path.append(_p)

import numpy as np
import ml_dtypes

import concourse.bass as bass
import concourse.mybir as mybir
import concourse.tile as tile
from concourse import bacc
from concourse.bass_utils import run_bass_kernel_spmd
import concourse.hw_specs as hw_specs

F32 = mybir.dt.float32
F16 = mybir.dt.float16
BF16 = mybir.dt.bfloat16
AL = mybir.AluOpType
AF = mybir.ActivationFunctionType

N_CORES = 8
D = 256
M = 100          # NUM_MIX * NUM_OUT
K = 10           # mixtures
C = 3            # channels
C2 = float(127.5 * np.exp(-7.0))   # eps folded into scales_t
KBIG = 30000.0                     # masked-shift magnitude (f16-exact)
N_ACT_RECIP = 4                    # tiles whose reciprocal runs on ACT


def _force_single_act_table():
    """All ACT funcs used here (Exp, Ln) live in natural_log_exp_and_others.
    The default chooser flip-flops between exp/ln sets, reloading tables
    (~1.3us each). Empty every other set (keeping dict order so set ids stay
    aligned with act_info.json) so one table load serves the whole kernel."""
    if getattr(hw_specs, "_ant_single_set", False):
        return
    orig = hw_specs.get_activation_tables
    import functools

    @functools.cache
    def patched(arch):
        tabs = dict(orig(arch))
        keep = "natural_log_exp_and_others"
        if keep in tabs:
            tabs = {k: (v if k == keep else set()) for k, v in tabs.items()}
        return tabs

    hw_specs.get_activation_tables = patched
    bacc.get_activation_tables = patched
    hw_specs._ant_single_set = True


def _v(ap0, offset, pattern):
    """AP on a tile's [:, :] AP: keep partition dim, replace free dims
    (stride-0 broadcast dims allowed)."""
    if not isinstance(ap0, bass.AP):
        ap0 = ap0[:, :]
    return bass.AP(tensor=ap0.tensor, offset=ap0.offset + offset,
                   ap=[list(ap0.ap[0])] + [list(p) for p in pattern])


def build_program(pix=16384, with_bias=False):
    """Single-core SPMD program. pix must be a multiple of 2048."""
    TP = 2048                  # pixels per macro-tile (2 images)
    NT = pix // TP             # macro-tiles
    NS = 16                    # 128-px subtiles per macro-tile
    NIMG = pix // 1024
    NKC = NS * K * C           # 480
    NK = NS * K                # 160

    _force_single_act_table()
    nc = bacc.Bacc("TRN2", target_bir_lowering=False, debug=False)

    xT_d = nc.dram_tensor("xT", [NT * D, TP], BF16, kind="ExternalInput").ap()
    w_d = nc.dram_tensor("w", [128, 2 * M], BF16, kind="ExternalInput").ap()
    vp_d = nc.dram_tensor("vp", [128, NT * NS * C], F16, kind="ExternalInput").ap()
    q_d = nc.dram_tensor("q", [128, NT * NS * 2], F16, kind="ExternalInput").ap()
    aux_d = nc.dram_tensor("aux", [128, NT * 3 * NKC], F16,
                           kind="ExternalInput").ap()
    if with_bias:
        bias_d = nc.dram_tensor("bias", [1, M], BF16, kind="ExternalInput").ap()
    acc_d = nc.dram_tensor("acc", [128, NIMG], F32, kind="ExternalOutput").ap()

    with tile.TileContext(nc) as tc, \
            tc.tile_pool(name="const", bufs=1) as cpool, \
            tc.tile_pool(name="xin", bufs=4) as xpool, \
            tc.tile_pool(name="ps", bufs=2, space="PSUM") as pspool, \
            tc.tile_pool(name="ep", bufs=2) as ep, \
            tc.tile_pool(name="ep3", bufs=3) as ep3, \
            tc.tile_pool(name="ep4", bufs=4) as ep4, \
            tc.tile_pool(name="ep5", bufs=5) as ep5:

        # DMA order matters for pipeline fill: w + first x tiles gate the
        # PE; aux is not needed until s2(0), so it goes last.
        w_sb = cpool.tile([128, 2 * M], BF16)
        nc.sync.dma_start(out=w_sb, in_=w_d)
        vp_sb = cpool.tile([128, NT * NS * C], F16)
        q_sb = cpool.tile([128, NT * NS * 2], F16)
        aux_sb = cpool.tile([128, NT * 3 * NKC], F16)
        acc = cpool.tile([128, NIMG], F32)
        zero_sb = cpool.tile([128, 2 * NKC], F32)
        nc.gpsimd.memset(zero_sb, 0.0)
        nl127_sb = cpool.tile([128, 1], F32)
        nc.gpsimd.memset(nl127_sb, float(-np.log(127.5)))
        if with_bias:
            bias_sb = cpool.tile([1, M], BF16)
            nc.sync.dma_start(out=bias_sb, in_=bias_d)
            ones_sb = cpool.tile([1, 128], BF16)
            nc.vector.memset(ones_sb, 1.0)

        # Stages s1b..s7 run once per PAIR of macro-tiles at double width,
        # halving instruction count + semaphore traffic on every engine.
        NP = NT // 2               # pairs
        NKC2 = 2 * NKC             # 960
        NK2 = 2 * NK               # 320
        st = [dict() for _ in range(NT)]     # per-tile (s1) state
        stp = [dict() for _ in range(NP)]    # per-pair state

        # AP dim patterns
        #  psum field view for field f0 at offset f, iterating [(c,) s, k]
        def fview(ps, f, cdim=False):
            dims = ([[1, C]] if cdim else []) + [[128, NS], [K, K]]
            return _v(ps[:, :], f, dims)

        # layout: per-tile epilogue tensors are c-major packed
        # (index = c*160 + s*10 + k); pair tensors are [half0 | half1] blocks

        def dma_tile(j):
            # both D-halves in one DMA: [xt0 | xt1] side by side
            xt = xpool.tile([128, 2 * TP], BF16, tag="xt")
            nc.sync.dma_start(out=xt, in_=bass.AP(
                tensor=xT_d.tensor, offset=j * D * TP,
                ap=[[TP, 128], [128 * TP, 2], [1, TP]]))
            st[j]["xt"] = xt

        def s1(j):
            t, h = j // 2, j & 1
            xt = st[j].pop("xt")
            xt0, xt1 = xt[:, 0:TP], xt[:, TP:2 * TP]
            ps = pspool.tile([128, 2048], F32, tag="ps")
            for sub in range(NS):
                o = ps[:, sub * 128:sub * 128 + M]
                if with_bias:
                    nc.tensor.matmul(o, ones_sb[:, :], bias_sb[:, :],
                                     start=True, stop=False)
                nc.tensor.matmul(o, xt0[:, sub * 128:(sub + 1) * 128],
                                 w_sb[:, 0:M], start=not with_bias, stop=False)
                nc.tensor.matmul(o, xt1[:, sub * 128:(sub + 1) * 128],
                                 w_sb[:, M:2 * M], start=False, stop=True)

            if h == 0:
                stp[t]["es"] = ep.tile([128, NKC2], F16, tag="es", name="es")
                stp[t]["epair"] = ep5.tile([128, 2 * NK2], F16, tag="epair",
                                           name="epair")
                stp[t]["f0s"] = ep4.tile([128, NK2], F32, tag="f0s",
                                         name="f0s")
                stp[t]["cc"] = ep.tile([128, NKC2], F16, tag="cc", name="cc")
                stp[t]["A"] = ep3.tile([128, NKC2], F16, tag="A", name="A")
            es = stp[t]["es"]
            epair = stp[t]["epair"]
            f0s = stp[t]["f0s"]
            cc = stp[t]["cc"]
            A = stp[t]["A"]
            # ACT: Es = exp(s_raw) into this tile's half, packed (s,k,c)
            nc.scalar.activation(es[:, h * NKC:(h + 1) * NKC],
                                 fview(ps, 4, True), AF.Exp)
            # ACT copies out of PSUM (GpSimd cannot touch PSUM on TRN2)
            nc.scalar.activation(f0s[:, h * NK:(h + 1) * NK], fview(ps, 0),
                                 AF.Copy)
            nc.scalar.activation(cc[:, h * NKC:(h + 1) * NKC],
                                 fview(ps, 7, True), AF.Copy)
            # DVE: A-half = locs' + vp2 (c-major packed)
            Ah = A[:, h * NKC:(h + 1) * NKC]
            vpb = _v(vp_sb[:, :], j * NS * C, [[1, C], [C, NS], [0, K]])
            nc.vector.tensor_tensor(Ah, fview(ps, 1, True), vpb, AL.add)

        def s1b(t):
            # Pool: pair-wide coupling products + adds into A channel slices
            cc = stp[t].pop("cc")
            A = stp[t]["A"]
            qb0 = _v(q_sb[:, :], t * 2 * NS * 2,
                     [[NS * 2, 2], [2, NS], [0, K]])
            qb1 = _v(q_sb[:, :], t * 2 * NS * 2 + 1,
                     [[NS * 2, 2], [2, NS], [0, K]])
            ccv = [_v(cc[:, :], c * NK, [[NKC, 2], [K, NS], [1, K]])
                   for c in range(C)]
            Av = [_v(A[:, :], c * NK, [[NKC, 2], [K, NS], [1, K]])
                  for c in range(C)]
            h0 = ep.tile([128, NK2], F16, tag="h0")
            h1 = ep.tile([128, NK2], F16, tag="h1")
            h2 = ep.tile([128, NK2], F16, tag="h2")
            hv = [_v(h[:, :], 0, [[NK, 2], [K, NS], [1, K]])
                  for h in (h0, h1, h2)]
            nc.gpsimd.tensor_tensor(hv[0], qb0, ccv[0], AL.mult)
            nc.gpsimd.tensor_tensor(hv[1], qb0, ccv[1], AL.mult)
            nc.gpsimd.tensor_tensor(hv[2], qb1, ccv[2], AL.mult)
            nc.gpsimd.tensor_tensor(Av[1], Av[1], hv[0], AL.add)
            nc.gpsimd.tensor_tensor(Av[2], Av[2], hv[1], AL.add)
            nc.gpsimd.tensor_tensor(Av[2], Av[2], hv[2], AL.add)
            es = stp[t].pop("es")
            # ls = softplus(s) in f32; r = 1/(127.5*ls) with the 127.5
            # folded into the f16 convert / exp bias (the +e^-7 epsilon is
            # dropped: softplus(s) >= 2e-3 for this input distribution, so
            # the shift is <5% on the rarest elements, ~2e-5 on the output)
            ls = ep.tile([128, NKC2], F32, tag="ls")
            nc.scalar.activation(ls, es, AF.Ln, bias=1.0)
            # r16 lands directly in xr's third block (the exp reads it there
            # unshifted; the mask acts multiplicatively on e^{-r} later)
            triple = ep.tile([128, 2 * NKC2], F16, tag="triple")
            xr = ep.tile([128, 3 * NKC2], F16, tag="xr")
            if t < N_ACT_RECIP:
                lr = ep.tile([128, NKC2], F32, tag="lr")
                nc.scalar.activation(lr, ls, AF.Ln)
                nc.scalar.activation(xr[:, 2 * NKC2:3 * NKC2], lr, AF.Exp,
                                     scale=-1.0, bias=nl127_sb[:, :])
            else:
                rr = ep.tile([128, NKC2], F32, tag="rr")
                nc.vector.reciprocal_approx_fast(rr, ls)
                nc.vector.tensor_scalar(xr[:, 2 * NKC2:3 * NKC2], rr,
                                        1.0 / 127.5, None, AL.mult)
            stp[t]["triple"] = triple
            stp[t]["xr"] = xr

        def s2(t):
            A = stp[t].pop("A")
            triple = stp[t].pop("triple")
            xr = stp[t].pop("xr")
            r16 = xr[:, 2 * NKC2:3 * NKC2]
            nc.vector.tensor_tensor(triple[:, 0:NKC2], A, r16, AL.mult)  # p0
            nc.vector.tensor_tensor(triple[:, NKC2:2 * NKC2], r16,
                                    triple[:, 0:NKC2], AL.subtract)      # nm0
            # masked shifts for [p', nm'] in one op (r is masked via gm)
            auxv = _v(aux_sb[:, :], t * 2 * 3 * NKC,
                      [[NKC, 2], [3 * NKC, 2], [1, NKC]])
            nc.vector.tensor_tensor(
                xr[:, 0:2 * NKC2].rearrange("p (b h e) -> p b h e", b=2, h=2),
                triple[:, :].rearrange("p (b h e) -> p b h e", b=2, h=2),
                auxv, AL.add)
            mnp = ep.tile([128, 2 * NKC2], F16, tag="mnp")
            nc.vector.tensor_scalar(mnp, xr[:, 0:2 * NKC2], 0.0, None, AL.min)
            # in-place |x| on the [p', nm'] part: clear the f16 sign bit
            u16 = xr[:, 0:2 * NKC2].bitcast(mybir.dt.uint16)
            nc.vector.tensor_scalar(u16, u16, 0x7FFF, None, AL.bitwise_and)
            # one exp over [|p'|, |nm'|, r]
            e012 = ep.tile([128, 3 * NKC2], F16, tag="e012")
            nc.scalar.activation(e012, xr, AF.Exp, scale=-1.0)
            # gm = -1 (normal) / 0 (masked): ln(1+gm*e^{-r}) gives L2, and
            # exactly 0 for boundary pixels
            gmv = _v(aux_sb[:, :], t * 2 * 3 * NKC + 2 * NKC,
                     [[3 * NKC, 2], [1, NKC]])
            nc.vector.tensor_tensor(
                e012[:, 2 * NKC2:3 * NKC2].rearrange("p (h e) -> p h e", h=2),
                e012[:, 2 * NKC2:3 * NKC2].rearrange("p (h e) -> p h e", h=2),
                gmv, AL.mult)
            l012 = ep.tile([128, 3 * NKC2], F16, tag="l012")
            nc.scalar.activation(l012, e012, AF.Ln, bias=1.0)
            stp[t]["mnp"] = mnp
            stp[t]["l012"] = l012

        def s3(t):
            mnp = stp[t].pop("mnp")
            l012 = stp[t].pop("l012")
            f0s = stp[t]["f0s"]
            uv = ep.tile([128, 2 * NKC2], F16, tag="uv")
            nc.vector.tensor_tensor(uv, l012[:, 0:2 * NKC2], mnp, AL.subtract)
            w1 = ep.tile([128, NKC2], F16, tag="w1")
            nc.vector.tensor_tensor(w1, uv[:, 0:NKC2], uv[:, NKC2:2 * NKC2],
                                    AL.add)
            wt = ep.tile([128, NKC2], F16, tag="wt")
            nc.vector.tensor_tensor(wt, w1, l012[:, 2 * NKC2:3 * NKC2],
                                    AL.subtract)
            # S = sum_c w: wt-pair is [h][c][s*k] blocks -> two adds
            wv = [_v(wt[:, :], c * NK, [[NKC, 2], [1, NK]]) for c in range(C)]
            S01 = ep.tile([128, NK2], F16, tag="S01")
            s01v = _v(S01[:, :], 0, [[NK, 2], [1, NK]])
            nc.vector.tensor_tensor(s01v, wv[0], wv[1], AL.add)
            S = ep.tile([128, NK2], F32, tag="S")
            nc.vector.tensor_tensor(_v(S[:, :], 0, [[NK, 2], [1, NK]]),
                                    s01v, wv[2], AL.add)
            z = ep.tile([128, NK2], F32, tag="z")
            nc.vector.scalar_tensor_tensor(z, S, -1.0, f0s, AL.mult, AL.add)
            m1n = ep5.tile([128, 2 * NS], F32, tag="m1n")
            nc.vector.tensor_reduce(m1n, _v(z[:, :], 0, [[K, 2 * NS], [1, K]]),
                                    axis=mybir.AxisListType.X, op=AL.max,
                                    negate=True)
            zz = ep.tile([128, NK2], F32, tag="zz")
            nc.vector.tensor_tensor(
                zz, z, _v(m1n[:, :], 0, [[1, 2 * NS], [0, K]]), AL.add)
            stp[t]["m1n"] = m1n
            stp[t]["zz"] = zz

        def s4(t):
            zz = stp[t].pop("zz")
            epair = stp[t].pop("epair")
            m1n = stp[t].pop("m1n")
            f0s = stp[t].pop("f0s")
            nc.scalar.activation(epair[:, NK2:2 * NK2], f0s, AF.Exp)
            nc.scalar.activation(epair[:, 0:NK2], zz, AF.Exp)
            s12 = ep.tile([128, 4 * NS], F32, tag="s12")
            nc.vector.reduce_sum(
                s12, _v(epair[:, :], 0, [[NK2, 2], [K, 2 * NS], [1, K]]),
                axis=mybir.AxisListType.X)
            lse = ep.tile([128, 4 * NS], F32, tag="lse")
            nc.scalar.activation(lse, s12, AF.Ln)
            dt = ep.tile([128, 2 * NS], F32, tag="dt")
            nc.vector.tensor_tensor(dt, lse[:, 0:2 * NS],
                                    lse[:, 2 * NS:4 * NS], AL.subtract)
            mx = ep.tile([128, 2 * NS], F32, tag="mx")
            nc.vector.tensor_tensor(mx, dt, m1n, AL.subtract)
            nc.vector.reduce_sum(acc[:, 4 * t:4 * t + 4],
                                 _v(mx[:, :], 0, [[8, 4], [1, 8]]),
                                 axis=mybir.AxisListType.X)
            nc.sync.dma_start(out=acc_d[:, 4 * t:4 * t + 4],
                              in_=acc[:, 4 * t:4 * t + 4])

        pair_stages = [s1b, s2, s3, s4]
        dma_tile(0)
        dma_tile(1)
        nc.sync.dma_start(out=vp_sb, in_=vp_d)
        nc.sync.dma_start(out=q_sb, in_=q_d)
        dma_tile(2)
        nc.sync.dma_start(out=aux_sb, in_=aux_d)
        # deepest stage first: each engine opens the iteration with work on
        # the oldest in-flight pair, whose cross-engine deps resolved in
        # earlier iterations. pair stage d fires at tile-iteration 2t+2+d.
        for i in range(NT + 1 + len(pair_stages)):
            if i + 3 < NT:
                dma_tile(i + 3)
            for d in range(len(pair_stages) - 1, -1, -1):
                ii = i - 2 - d
                if ii >= 0 and ii % 2 == 0 and ii // 2 < NP:
                    pair_stages[d](ii // 2)
            if i < NT:
                s1(i)


    nc.compile()
    return nc


_CACHE = {}


def _get_program(pix, with_bias):
    key = (pix, with_bias)
    if key not in _CACHE:
        _CACHE[key] = build_program(pix, with_bias)
    return _CACHE[key]


def _prescale_w(W_conv):
    Wr = W_conv.astype(np.float64).reshape(D, K, 10).copy()
    Wr[:, :, 1:4] *= -127.5
    Wr[:, :, 7:10] *= -1.0
    Wr = Wr.reshape(D, M)
    # pre-pack to the SBUF layout [128, 2*M]: w_sb[p, h*M+m] = W'[h*128+p, m]
    wsb = Wr.reshape(2, 128, M).transpose(1, 0, 2).reshape(128, 2 * M)
    return np.ascontiguousarray(wsb.astype(ml_dtypes.bfloat16))


def _pack_host(vf, per):
    """vf [per,3] raw 0..255 -> (vp, q, aux) host tensors for one core.

    Pixel local index = j*2048 + s*128 + partition; partition-major packing:
      vp  [128, NT*NS*C]   : vp2 = value-127            at j*48 + s*3 + c
      q   [128, NT*NS*2]   : vp2_{0,1} - 0.5            at j*32 + s*2 + d
      aux [128, NT*3*480]  : (PK,MK,RK) blocks, each c-major packed
                             at j*1440 + b*480 + c*160 + s*10 + k (k-bcast)
    """
    NT = per // 2048
    v = vf.reshape(NT, NS_G, 128, C).transpose(2, 0, 1, 3)  # [128, NT, s, c]
    vp2 = v - 127.0
    vp = np.ascontiguousarray(
        vp2.reshape(128, -1).astype(np.float16))
    q = np.ascontiguousarray(
        (vp2[..., 0:2] - 0.5).reshape(128, -1).astype(np.float16))
    mlow = (v == 0.0)
    mhigh = (v == 255.0)
    pk = KBIG * mhigh
    mk = KBIG * mlow
    gm = np.where(mlow | mhigh, 0.0, -1.0)
    aux = np.stack([pk, mk, gm], axis=2)            # [128, NT, b, s, c]
    aux = np.transpose(aux, (0, 1, 2, 4, 3))        # [128, NT, b, c, s]
    aux = np.broadcast_to(aux[..., None],
                          aux.shape + (K,))         # [128, NT, b, c, s, k]
    aux = np.ascontiguousarray(
        aux.reshape(128, -1).astype(np.float16))
    return vp, q, aux


NS_G = 16


def shard_inputs(x, value, W_conv, b_conv, n_cores=N_CORES):
    B = x.shape[0]
    pix_total = B * x.shape[1] * x.shape[2]
    per = pix_total // n_cores
    xf = np.ascontiguousarray(x.reshape(pix_total, D).astype(np.float32))
    vf = value.reshape(pix_total, C).astype(np.float32)
    w_bf = _prescale_w(np.asarray(W_conv))
    with_bias = bool(np.any(b_conv))
    in_maps = []
    for i in range(n_cores):
        xT = xf[i * per:(i + 1) * per].T.astype(ml_dtypes.bfloat16)
        # tile-contiguous layout [NT*D, 2048] so each tile DMA is one
        # dense block
        xT = np.ascontiguousarray(
            xT.reshape(D, per // 2048, 2048).transpose(1, 0, 2)
        ).reshape(-1, 2048)
        vp, q, aux = _pack_host(vf[i * per:(i + 1) * per], per)
        mm = {"xT": xT, "w": w_bf, "vp": vp, "q": q, "aux": aux}
        if with_bias:
            br = b_conv.astype(np.float64).reshape(K, 10).copy()
            br[:, 1:4] *= -127.5
            br[:, 7:10] *= -1.0
            mm["bias"] = br.reshape(1, M).astype(ml_dtypes.bfloat16)
        in_maps.append(mm)
    return in_maps, with_bias, per


def kernel(x, value, W_conv, b_conv):
    x = np.asarray(x)
    value = np.asarray(value)
    W_conv = np.asarray(W_conv)
    b_conv = np.asarray(b_conv)
    in_maps, with_bias, per = shard_inputs(x, value, W_conv, b_conv)
    nc = _get_program(per, with_bias)
    res = run_bass_kernel_spmd(nc, in_maps, list(range(N_CORES)))
    parts = []
    for i in range(N_CORES):
        acc = res.results[i]["acc"]
        parts.append(acc.astype(np.float64).sum(axis=0).astype(np.float32))
    return np.concatenate(parts)
